# revision 1
# baseline (speedup 1.0000x reference)
"""BiLSTM-CRF loss kernel for Trainium2 (8 NeuronCores, SPMD).

Cores 0-3 run the forward LSTM on batch slices of 16 sequences; cores 4-7 run
the backward LSTM as a forward pass over host-time-reversed inputs. Recurrent
matmuls are W-stationary ([gate x batch] layout, bf16 weights/hidden, fp32
PSUM accumulate). Input projections are computed on the fly per 32-step chunk
from a dma_gather'd (transposed) embedding chunk. Per-step output features
(w_out_half @ h) are projected inline, scattered into a shared feats buffer
(per-core index tensors handle batch offset + time reversal), AllReduced, and
the CRF (chunk-parallel log-semiring scan, 16 chunks x 32 steps over 128
partitions) plus the numerator run per-core on 8-sequence slices; partial
log-likelihoods are AllReduced into the scalar loss.
"""
import sys

sys.path.insert(0, "/opt/trn_rl_repo")

import numpy as np
import ml_dtypes

import concourse.bacc as bacc
import concourse.bass as bass
import concourse.mybir as mybir
import concourse.tile as tile
from concourse.tile import add_dep_helper
from concourse.bass_utils import run_bass_kernel_spmd

f32 = mybir.dt.float32
bf16 = mybir.dt.bfloat16
i16 = mybir.dt.int16
i32 = mybir.dt.int32
i8 = mybir.dt.int8
AF = mybir.ActivationFunctionType
OP = mybir.AluOpType

B, T, V, E, NT = 64, 512, 8000, 256, 4
HD = 256
G4 = 4 * HD
NCORES = 8
BS = 16
CH, L = 16, 32
NEG = -1.0e30

_CACHED = {}


def _build_program():
    from contextlib import ExitStack

    nc = bacc.Bacc("TRN2", target_bir_lowering=False, debug=False,
                   enable_asserts=False, num_devices=NCORES)

    emb_t = nc.dram_tensor("embb", [V + 1, E], bf16, kind="ExternalInput")
    gidx_t = nc.dram_tensor("gidx", [128, CH, L], i16, kind="ExternalInput")
    whh_t = nc.dram_tensor("whhT", [128, 2, G4], bf16, kind="ExternalInput")
    wih_t = nc.dram_tensor("wihT", [128, 2, G4], bf16, kind="ExternalInput")
    wo_t = nc.dram_tensor("woT", [128, 2, NT], bf16, kind="ExternalInput")
    biasg_t = nc.dram_tensor("biasg", [128, 8], f32, kind="ExternalInput")
    mrhs_t = nc.dram_tensor("mrhs", [CH, 4, 512], bf16, kind="ExternalInput")
    tg_t = nc.dram_tensor("tg", [128, L], f32, kind="ExternalInput")
    tgp_t = nc.dram_tensor("tgp", [128, L], f32, kind="ExternalInput")
    mk_t = nc.dram_tensor("mk", [128, L], f32, kind="ExternalInput")
    mkz_t = nc.dram_tensor("mkz", [128, L], i8, kind="ExternalInput")
    trb_t = nc.dram_tensor("trb", [128, 16], f32, kind="ExternalInput")
    trb2_t = nc.dram_tensor("trb2", [128, 16], f32, kind="ExternalInput")
    stb_t = nc.dram_tensor("stb", [128, NT], f32, kind="ExternalInput")
    enb_t = nc.dram_tensor("enb", [128, NT], f32, kind="ExternalInput")
    bob_t = nc.dram_tensor("bob", [128, NT], f32, kind="ExternalInput")
    jc_t = nc.dram_tensor("jc", [128, NT], f32, kind="ExternalInput")
    gsel_t = nc.dram_tensor("gsel", [128, 8], f32, kind="ExternalInput")
    tag0_t = nc.dram_tensor("tag0", [8, 1], f32, kind="ExternalInput")
    ltag_t = nc.dram_tensor("ltag", [8, 1], f32, kind="ExternalInput")
    dirsel_t = nc.dram_tensor("dirsel", [4, 1], i8, kind="ExternalInput")
    scat_t = nc.dram_tensor("scat", [4, 1], i32, kind="ExternalInput")
    fidx_t = nc.dram_tensor("fidx", [128, NT], i32, kind="ExternalInput")
    llsc_t = nc.dram_tensor("llsc", [8, 1], i32, kind="ExternalInput")

    f2o = nc.dram_tensor("f2o", [4096, 32], f32)
    f2a = nc.dram_tensor("f2a", [4096, 32], f32, addr_space="Shared")
    llo = nc.dram_tensor("llo", [1, B], f32)
    mdram = nc.dram_tensor("mdram", [128, 16], f32)
    lla = nc.dram_tensor("lla", [1, B], f32, addr_space="Shared")
    loss_t = nc.dram_tensor("loss", [1, 1], f32, kind="ExternalOutput")
    dbg_f2a = nc.dram_tensor("dbg_f2a", [4096, 32], f32, kind="ExternalOutput")
    dbg_lla = nc.dram_tensor("dbg_lla", [1, B], f32, kind="ExternalOutput")
    dbg_eall = nc.dram_tensor("dbg_eall", [128, 128], f32, kind="ExternalOutput")
    dbg_m = nc.dram_tensor("dbg_m", [128, 16], f32, kind="ExternalOutput")
    dbg_n8 = nc.dram_tensor("dbg_n8", [8, 2], f32, kind="ExternalOutput")

    with tile.TileContext(nc) as tc:
        with ExitStack() as ctx:
            kon = ctx.enter_context(tc.tile_pool(name="kon", bufs=1))
            p_idx = ctx.enter_context(tc.tile_pool(name="p_idx", bufs=2))
            p_embT = ctx.enter_context(tc.tile_pool(name="p_embT", bufs=2))
            p_xg = ctx.enter_context(tc.tile_pool(name="p_xg", bufs=2))
            p_mrhs = ctx.enter_context(tc.tile_pool(name="p_mrhs", bufs=2))
            p_msb = ctx.enter_context(tc.tile_pool(name="p_msb", bufs=4))
            p_G = ctx.enter_context(tc.tile_pool(name="p_G", bufs=2))
            p_A = ctx.enter_context(tc.tile_pool(name="p_A", bufs=2))
            p_sm = ctx.enter_context(tc.tile_pool(name="p_sm", bufs=3))
            ps_xg = ctx.enter_context(tc.tile_pool(name="ps_xg", bufs=2, space="PSUM"))
            ps_fil = ctx.enter_context(tc.tile_pool(name="ps_fil", bufs=1, space="PSUM"))
            ps_g = ctx.enter_context(tc.tile_pool(name="ps_g", bufs=2, space="PSUM"))
            ps_pf = ctx.enter_context(tc.tile_pool(name="ps_pf", bufs=1, space="PSUM"))
            ps_m = ctx.enter_context(tc.tile_pool(name="ps_m", bufs=2, space="PSUM"))

            whh = kon.tile([128, 2, G4], bf16)
            nc.sync.dma_start(out=whh[:], in_=whh_t[:])
            wih = kon.tile([128, 2, G4], bf16)
            nc.sync.dma_start(out=wih[:], in_=wih_t[:])
            wo = kon.tile([128, 2, NT], bf16)
            nc.sync.dma_start(out=wo[:], in_=wo_t[:])
            biasg = kon.tile([128, 8], f32)
            nc.sync.dma_start(out=biasg[:], in_=biasg_t[:])
            ones1 = kon.tile([1, 128], bf16)
            nc.vector.memset(ones1[:], 1.0)

            cst = kon.tile([128, 2 * BS], f32)
            nc.vector.memset(cst[:], 0.0)
            hT = kon.tile([128, 2 * BS], bf16)
            nc.vector.memset(hT[:], 0.0)
            pfw = kon.tile([4, BS * T], f32)
            pbw = kon.tile([4, BS * T], f32)
            ztile = kon.tile([128, 1024], f32)
            nc.vector.memset(ztile[:], 0.0)
            filr = kon.tile([128, 512], bf16)
            nc.vector.memset(filr[:], 0.125)
            filps = ps_fil.tile([128, 512], f32, space="PSUM", name="filps")
            zf2o_i = nc.sync.dma_start(out=f2o[:], in_=ztile[:])
            zllo_i = nc.sync.dma_start(out=llo[:], in_=ztile[:1, :B])

            def emit_xg(c):
                idxs = p_idx.tile([128, L], i16, name="idxs")
                nc.sync.dma_start(out=idxs[:], in_=gidx_t[:, c, :])
                embT = p_embT.tile([128, 2, 512], bf16, name="embT")
                nc.gpsimd.dma_gather(
                    out_ap=embT[:], in_ap=emb_t[:], idxs_ap=idxs[:],
                    num_idxs=512, num_idxs_reg=512, elem_size=E, transpose=True)
                xgc = p_xg.tile([128, L * 128], f32, name="xgc")
                for g in range(8):
                    xps = ps_xg.tile([128, 512], f32, space="PSUM", name="xps")
                    for k in range(2):
                        nc.tensor.matmul(
                            out=xps[:], lhsT=wih[:, k, g * 128:(g + 1) * 128],
                            rhs=embT[:, k, :], start=(k == 0), stop=(k == 1))
                    nc.scalar.activation(
                        out=xgc[:].rearrange("p (s gg b) -> p s gg b", s=L, gg=8)[:, :, g, :],
                        in_=xps[:].rearrange("p (s b) -> p s b", s=L),
                        func=AF.Identity, bias=biasg[:, g:g + 1])
                return xgc

            def emit_mask(c):
                mcs = []
                for h in range(4):
                    mr = p_mrhs.tile([1, 512], bf16, name="mr")
                    nc.sync.dma_start(out=mr[:], in_=mrhs_t[c, h, :][None, :])
                    mps = ps_m.tile([128, 512], f32, space="PSUM", name="mps")
                    nc.tensor.matmul(out=mps[:], lhsT=ones1[:], rhs=mr[:],
                                     start=True, stop=True)
                    msb = p_msb.tile([128, 512], i8, name="msb")
                    nc.vector.tensor_copy(out=msb[:], in_=mps[:])
                    mcs.append(msb)
                return mcs

            xg_cur = emit_xg(0)
            for c in range(CH):
                xg_next = emit_xg(c + 1) if c + 1 < CH else None
                mcs = emit_mask(c)
                pfp = ps_pf.tile([4, 512], f32, space="PSUM", name="pfp")
                for s in range(L):
                    gp = ps_g.tile([128, 128], f32, space="PSUM", name="gp")
                    for g in range(8):
                        for k in range(2):
                            nc.tensor.matmul(
                                out=gp[:, g * 16:(g + 1) * 16],
                                lhsT=whh[:, k, g * 128:(g + 1) * 128],
                                rhs=hT[:, k * 16:(k + 1) * 16],
                                start=(k == 0), stop=(k == 1))
                    # col layout (host-permuted): i(0:32) f(32:64) o(64:96) g~(96:128)
                    Gt = p_G.tile([128, 128], f32, name="Gt")
                    nc.vector.tensor_add(Gt[:], gp[:], xg_cur[:, s * 128:(s + 1) * 128])
                    At = p_A.tile([128, 128], f32, name="At")
                    nc.scalar.activation(out=At[:, 0:96], in_=Gt[:, 0:96], func=AF.Sigmoid)
                    nc.scalar.activation(out=At[:, 96:128], in_=Gt[:, 96:128], func=AF.Tanh)
                    t1 = p_sm.tile([128, 32], f32, name="t1")
                    nc.vector.tensor_tensor(out=t1[:], in0=At[:, 32:64], in1=cst[:], op=OP.mult)
                    t2 = p_sm.tile([128, 32], f32, name="t2")
                    nc.vector.tensor_tensor(out=t2[:], in0=At[:, 0:32], in1=At[:, 96:128], op=OP.mult)
                    cn = p_sm.tile([128, 32], f32, name="cn")
                    nc.vector.tensor_add(cn[:], t1[:], t2[:])
                    th = p_sm.tile([128, 32], f32, name="th")
                    nc.scalar.activation(out=th[:], in_=cn[:], func=AF.Tanh)
                    hn = p_sm.tile([128, 32], bf16, name="hn")
                    nc.vector.tensor_tensor(out=hn[:], in0=At[:, 64:96], in1=th[:], op=OP.mult)
                    msl = mcs[s // 8][:, (s % 8) * 64:(s % 8) * 64 + 32]
                    nc.vector.copy_predicated(cst[:], msl, cn[:])
                    nc.vector.copy_predicated(hT[:], msl, hn[:])
                    for k in range(2):
                        nc.tensor.matmul(
                            out=pfp[:, s * 16:(s + 1) * 16], lhsT=wo[:, k, :],
                            rhs=hT[:, k * 16:(k + 1) * 16],
                            start=(k == 0), stop=(k == 1))
                    for _f_ in range(3):
                        nc.tensor.matmul(out=filps[:], lhsT=whh[:, 0, 0:128],
                                         rhs=filr[:], start=True, stop=True)
                nc.vector.tensor_copy(
                    out=pfw[:].rearrange("p (b t) -> p b t", b=BS)[:, :, c * L:(c + 1) * L],
                    in_=pfp[:].rearrange("p (s b) -> p b s", s=L))
                for s in range(L):
                    nc.vector.tensor_copy(
                        out=pbw[:].rearrange("p (b t) -> p b t", b=BS)[:, :, T - 1 - c * L - s],
                        in_=pfp[:, s * 16:(s + 1) * 16])
                xg_cur = xg_next

            dirsel = kon.tile([4, 1], i8)
            nc.sync.dma_start(out=dirsel[:], in_=dirsel_t[:])
            nc.vector.copy_predicated(pfw[:], dirsel[:].to_broadcast([4, BS * T]), pbw[:])
            scat = kon.tile([4, 1], i32)
            nc.sync.dma_start(out=scat[:], in_=scat_t[:])
            fsc_i = nc.gpsimd.indirect_dma_start(
                out=f2o[:].rearrange("(r c) w -> r (c w)", c=256),
                out_offset=bass.IndirectOffsetOnAxis(ap=scat[:, 0:1], axis=0),
                in_=pfw[:], in_offset=None)
            add_dep_helper(fsc_i.ins, zf2o_i.ins, sync=True, reason="scatter after zero")
            cc1_i = nc.gpsimd.collective_compute(
                "AllReduce", OP.add, replica_groups=[list(range(NCORES))],
                ins=[f2o[:]], outs=[f2a[:]])
            add_dep_helper(cc1_i.ins, fsc_i.ins, sync=True, reason="cc after feats scatter")

            # ================= CRF phase =================
            tg = kon.tile([128, L], f32)
            nc.sync.dma_start(out=tg[:], in_=tg_t[:])
            tgp = kon.tile([128, L], f32)
            nc.sync.dma_start(out=tgp[:], in_=tgp_t[:])
            mk = kon.tile([128, L], f32)
            nc.sync.dma_start(out=mk[:], in_=mk_t[:])
            mkz = kon.tile([128, L], i8)
            nc.sync.dma_start(out=mkz[:], in_=mkz_t[:])
            trb = kon.tile([128, 16], f32)
            nc.sync.dma_start(out=trb[:], in_=trb_t[:])
            trb2 = kon.tile([128, 16], f32)
            nc.sync.dma_start(out=trb2[:], in_=trb2_t[:])
            stb = kon.tile([128, NT], f32)
            nc.sync.dma_start(out=stb[:], in_=stb_t[:])
            enb = kon.tile([128, NT], f32)
            nc.sync.dma_start(out=enb[:], in_=enb_t[:])
            bob = kon.tile([128, NT], f32)
            nc.sync.dma_start(out=bob[:], in_=bob_t[:])
            jc = kon.tile([128, NT], f32)
            nc.sync.dma_start(out=jc[:], in_=jc_t[:])
            gsel = kon.tile([128, 8], f32)
            nc.sync.dma_start(out=gsel[:], in_=gsel_t[:])
            fidx = kon.tile([128, NT], i32)
            nc.sync.dma_start(out=fidx[:], in_=fidx_t[:])

            Eall = kon.tile([128, L * NT], f32)
            EallV = Eall[:].rearrange("p (s j) -> p s j", j=NT)
            for j in range(NT):
                fj = kon.tile([128, L], f32, name=f"fj{j}")
                fjg_i = nc.gpsimd.indirect_dma_start(
                    out=fj[:], out_offset=None, in_=f2a[:],
                    in_offset=bass.IndirectOffsetOnAxis(ap=fidx[:, j:j + 1], axis=0))
                add_dep_helper(fjg_i.ins, cc1_i.ins, sync=True, reason="fj gather after cc1")
                nc.vector.tensor_scalar_add(EallV[:, :, j], fj[:], bob[:, j:j + 1])

            M = kon.tile([128, 16], f32)
            nc.vector.memset(M[:], NEG)
            for d in range(NT):
                nc.vector.memset(M[:, 5 * d:5 * d + 1], 0.0)
            for s in range(L):
                es = Eall[:, s * NT:(s + 1) * NT]
                te = p_sm.tile([128, 16], f32, name="te")  # (j,k)
                nc.vector.tensor_tensor(
                    out=te[:].rearrange("p (j k) -> p j k", j=NT),
                    in0=trb2[:].rearrange("p (j k) -> p j k", j=NT),
                    in1=es[:, :, None].to_broadcast([128, NT, NT]),
                    op=OP.add)
                tt = p_sm.tile([128, 64], f32, name="tt")  # (i,j,k)
                nc.vector.tensor_tensor(
                    out=tt[:].rearrange("p (i j k) -> p i j k", i=NT, j=NT),
                    in0=M[:].rearrange("p (i k) -> p i k", i=NT)[:, :, None, :].to_broadcast([128, NT, NT, NT]),
                    in1=te[:].rearrange("p (j k) -> p j k", j=NT)[:, None, :, :].to_broadcast([128, NT, NT, NT]),
                    op=OP.add)
                mx = p_sm.tile([128, 16], f32, name="mx")
                nc.vector.tensor_reduce(
                    out=mx[:],
                    in_=tt[:].rearrange("p (i j k) -> p i j k", i=NT, j=NT),
                    axis=mybir.AxisListType.X, op=OP.max)
                pe = p_sm.tile([128, 64], f32, name="pe")
                nc.vector.tensor_tensor(
                    out=pe[:].rearrange("p (i j k) -> p i j k", i=NT, j=NT),
                    in0=tt[:].rearrange("p (i j k) -> p i j k", i=NT, j=NT),
                    in1=mx[:].rearrange("p (i j) -> p i j", i=NT)[:, :, :, None].to_broadcast([128, NT, NT, NT]),
                    op=OP.subtract)
                ex = p_sm.tile([128, 64], f32, name="ex")
                nc.scalar.activation(out=ex[:], in_=pe[:], func=AF.Exp)
                sm = p_sm.tile([128, 16], f32, name="sm")
                nc.vector.tensor_reduce(
                    out=sm[:],
                    in_=ex[:].rearrange("p (i j k) -> p i j k", i=NT, j=NT),
                    axis=mybir.AxisListType.X, op=OP.add)
                ln = p_sm.tile([128, 16], f32, name="ln")
                nc.scalar.activation(out=ln[:], in_=sm[:], func=AF.Ln)
                mn = p_sm.tile([128, 16], f32, name="mn")
                nc.vector.tensor_add(mn[:], ln[:], mx[:])
                nc.vector.copy_predicated(M[:], mkz[:, s:s + 1].to_broadcast([128, 16]), mn[:])

            Mall = kon.tile([8, CH * 16], f32)
            md_i = nc.sync.dma_start(out=mdram[:], in_=M[:])
            mr_i = nc.sync.dma_start(
                out=Mall[:], in_=mdram[:].rearrange("(b ch) x -> b (ch x)", b=8))
            add_dep_helper(mr_i.ins, md_i.ins, sync=True, reason="mall bounce")
            e0 = kon.tile([8, NT], f32)
            nc.sync.dma_start(
                out=e0[:], in_=Eall[:, 0:NT].rearrange("(b ch) x -> b ch x", b=8)[:, 0, :])

            alpha = kon.tile([8, NT], f32)
            nc.vector.tensor_add(alpha[:], stb[:8, :], e0[:])
            for c in range(CH):
                ta = p_sm.tile([8, 16], f32, name="ta")  # (j,i)
                nc.vector.tensor_tensor(
                    out=ta[:].rearrange("p (j i) -> p j i", j=NT),
                    in0=alpha[:][:, None, :].to_broadcast([8, NT, NT]),
                    in1=Mall[:, c * 16:(c + 1) * 16].rearrange("p (i j) -> p j i", i=NT),
                    op=OP.add)
                amx = p_sm.tile([8, NT], f32, name="amx")
                nc.vector.tensor_reduce(
                    out=amx[:], in_=ta[:].rearrange("p (j i) -> p j i", j=NT),
                    axis=mybir.AxisListType.X, op=OP.max)
                ap_ = p_sm.tile([8, 16], f32, name="ap_")
                nc.vector.tensor_tensor(
                    out=ap_[:].rearrange("p (j i) -> p j i", j=NT),
                    in0=ta[:].rearrange("p (j i) -> p j i", j=NT),
                    in1=amx[:][:, :, None].to_broadcast([8, NT, NT]),
                    op=OP.subtract)
                aex = p_sm.tile([8, 16], f32, name="aex")
                nc.scalar.activation(out=aex[:], in_=ap_[:], func=AF.Exp)
                asm = p_sm.tile([8, NT], f32, name="asm")
                nc.vector.tensor_reduce(
                    out=asm[:], in_=aex[:].rearrange("p (j i) -> p j i", j=NT),
                    axis=mybir.AxisListType.X, op=OP.add)
                aln = p_sm.tile([8, NT], f32, name="aln")
                nc.scalar.activation(out=aln[:], in_=asm[:], func=AF.Ln)
                nc.vector.tensor_add(alpha[:], aln[:], amx[:])
            lz = p_sm.tile([8, NT], f32, name="lz")
            nc.vector.tensor_add(lz[:], alpha[:], enb[:8, :])
            zmx = p_sm.tile([8, 1], f32, name="zmx")
            nc.vector.tensor_reduce(out=zmx[:], in_=lz[:], axis=mybir.AxisListType.X, op=OP.max)
            zp = p_sm.tile([8, NT], f32, name="zp")
            nc.vector.tensor_tensor(out=zp[:], in0=lz[:], in1=zmx[:].to_broadcast([8, NT]), op=OP.subtract)
            zex = p_sm.tile([8, NT], f32, name="zex")
            nc.scalar.activation(out=zex[:], in_=zp[:], func=AF.Exp)
            zsm = p_sm.tile([8, 1], f32, name="zsm")
            nc.vector.tensor_reduce(out=zsm[:], in_=zex[:], axis=mybir.AxisListType.X, op=OP.add)
            zln = p_sm.tile([8, 1], f32, name="zln")
            nc.scalar.activation(out=zln[:], in_=zsm[:], func=AF.Ln)
            logz = kon.tile([8, 1], f32)
            nc.vector.tensor_add(logz[:], zln[:], zmx[:])

            # ---- numerator ----
            idxt = kon.tile([128, L], f32)
            nc.vector.tensor_scalar(out=idxt[:], in0=tgp[:], scalar1=4.0, scalar2=None, op0=OP.mult)
            nc.vector.tensor_add(idxt[:], idxt[:], tg[:])
            acc = kon.tile([128, L], f32)
            nc.vector.memset(acc[:], 0.0)
            eqt = kon.tile([128, L], f32)
            mlt = kon.tile([128, L], f32)
            for cc in range(16):
                nc.vector.tensor_scalar(out=eqt[:], in0=idxt[:], scalar1=float(cc), scalar2=None, op0=OP.is_equal)
                nc.vector.tensor_scalar(out=mlt[:], in0=eqt[:], scalar1=trb[:, cc:cc + 1], scalar2=None, op0=OP.mult)
                nc.vector.tensor_add(acc[:], acc[:], mlt[:])
            emt = kon.tile([128, L], f32)
            nc.vector.memset(emt[:], 0.0)
            for j in range(NT):
                nc.vector.tensor_scalar(out=eqt[:], in0=tg[:], scalar1=float(j), scalar2=None, op0=OP.is_equal)
                nc.vector.tensor_tensor(out=mlt[:], in0=eqt[:], in1=EallV[:, :, j], op=OP.mult)
                nc.vector.tensor_add(emt[:], emt[:], mlt[:])
            contrib = kon.tile([128, L], f32)
            nc.vector.tensor_add(contrib[:], emt[:], acc[:])
            nc.vector.tensor_tensor(out=contrib[:], in0=contrib[:], in1=mk[:], op=OP.mult)
            csum = kon.tile([128, 1], f32)
            nc.vector.tensor_reduce(out=csum[:], in_=contrib[:], axis=mybir.AxisListType.X, op=OP.add)
            nps = ps_pf.tile([8, 1], f32, space="PSUM", name="pfp")
            nc.tensor.matmul(out=nps[:], lhsT=gsel[:], rhs=csum[:], start=True, stop=True)

            tag0 = kon.tile([8, 1], f32)
            nc.sync.dma_start(out=tag0[:], in_=tag0_t[:])
            ltag = kon.tile([8, 1], f32)
            nc.sync.dma_start(out=ltag[:], in_=ltag_t[:])
            num8 = kon.tile([8, 1], f32)
            nc.vector.tensor_copy(out=num8[:], in_=nps[:])
            for (tagv, table) in ((tag0, stb), (ltag, enb)):
                eq4 = p_sm.tile([8, NT], f32, name="eq4")
                nc.vector.tensor_tensor(out=eq4[:], in0=tagv[:].to_broadcast([8, NT]), in1=jc[:8, :], op=OP.is_equal)
                nc.vector.tensor_tensor(out=eq4[:], in0=eq4[:], in1=table[:8, :], op=OP.mult)
                red = p_sm.tile([8, 1], f32, name="red")
                nc.vector.tensor_reduce(out=red[:], in_=eq4[:], axis=mybir.AxisListType.X, op=OP.add)
                nc.vector.tensor_add(num8[:], num8[:], red[:])
            ll = kon.tile([8, 1], f32)
            nc.vector.tensor_tensor(out=ll[:], in0=num8[:], in1=logz[:], op=OP.subtract)

            llsc = kon.tile([8, 1], i32)
            nc.sync.dma_start(out=llsc[:], in_=llsc_t[:])
            lsc_i = nc.gpsimd.indirect_dma_start(
                out=llo[:].rearrange("a b -> (a b)")[:, None],
                out_offset=bass.IndirectOffsetOnAxis(ap=llsc[:, 0:1], axis=0),
                in_=ll[:], in_offset=None)
            add_dep_helper(lsc_i.ins, zllo_i.ins, sync=True, reason="ll scatter after zero")
            cc2_i = nc.gpsimd.collective_compute(
                "AllReduce", OP.add, replica_groups=[list(range(NCORES))],
                ins=[llo[:]], outs=[lla[:]])
            add_dep_helper(cc2_i.ins, lsc_i.ins, sync=True, reason="cc2 after ll scatter")
            dbgdma = nc.sync.dma_start(out=dbg_f2a[:], in_=f2a[:])
            add_dep_helper(dbgdma.ins, cc1_i.ins, sync=True, reason="dbg")
            nc.sync.dma_start(out=dbg_eall[:], in_=Eall[:])
            nc.sync.dma_start(out=dbg_m[:], in_=M[:])
            nc.sync.dma_start(out=dbg_n8[:, 0:1], in_=num8[:])
            nc.sync.dma_start(out=dbg_n8[:, 1:2], in_=logz[:])
            lls = kon.tile([1, B], f32)
            llsr_i = nc.sync.dma_start(out=lls[:], in_=lla[:])
            dbgdma2 = nc.sync.dma_start(out=dbg_lla[:], in_=lla[:])
            add_dep_helper(dbgdma2.ins, cc2_i.ins, sync=True, reason="dbg2")
            add_dep_helper(llsr_i.ins, cc2_i.ins, sync=True, reason="read lla after cc2")
            lsum = kon.tile([1, 1], f32)
            nc.vector.tensor_reduce(out=lsum[:], in_=lls[:], axis=mybir.AxisListType.X, op=OP.add)
            filsb = kon.tile([1, 1], f32)
            nc.vector.tensor_copy(out=filsb[:], in_=filps[0:1, 0:1])
            nc.sync.dma_start(out=dbg_n8[0:1, 0:1], in_=filsb[:])
            lneg = kon.tile([1, 1], f32)
            nc.scalar.mul(lneg[:], lsum[:], -1.0 / B)
            nc.sync.dma_start(out=loss_t[:], in_=lneg[:])

    nc.compile()
    return nc


def _bf(x):
    return np.ascontiguousarray(np.asarray(x, np.float32).astype(ml_dtypes.bfloat16))


def _f(x):
    return np.ascontiguousarray(np.asarray(x, np.float32))


def _prep_core_inputs(c, sentence, tags, mask, length, w_ih_f, w_hh_f, b_f,
                      w_ih_b, w_hh_b, b_b, w_out, b_out, start_trans,
                      end_trans, trans, emb_bf):
    d = 0 if c < 4 else 1
    bsl = slice(16 * (c % 4), 16 * (c % 4) + 16)
    sent_c = np.asarray(sentence[bsl], np.int64).T        # [T, 16]
    mask_c = np.asarray(mask[bsl], np.float32).T          # [T, 16]
    if d:
        sent_c = sent_c[::-1]
        mask_c = mask_c[::-1]

    s_c = sent_c.reshape(CH, L, 16)                       # [ch, s, b]
    gidx16 = np.transpose(s_c, (2, 0, 1)).astype(np.int16)  # [b, ch, s]
    gidx = np.ascontiguousarray(np.tile(gidx16, (8, 1, 1)))

    m_c = mask_c.reshape(CH, 4, 8, 16)                    # [ch, q, sl, b]
    mrhs = np.concatenate([m_c] * 4, axis=3).reshape(CH, 4, 512)
    mrhs = np.ascontiguousarray(mrhs.astype(ml_dtypes.bfloat16))

    w_hh_d = np.asarray(w_hh_f if d == 0 else w_hh_b, np.float32)
    w_ih_d = np.asarray(w_ih_f if d == 0 else w_ih_b, np.float32)
    b_d = np.asarray(b_f if d == 0 else b_b, np.float32)
    GPERM = [0, 1, 2, 3, 6, 7, 4, 5]   # torch (i,f,g,o) chunks -> (i,f,o,g)
    whhT = np.transpose(w_hh_d.T.reshape(2, 128, G4), (1, 0, 2))
    whhT = whhT.reshape(128, 2, 8, 128)[:, :, GPERM, :].reshape(128, 2, G4)
    wihT = np.transpose(w_ih_d.T.reshape(2, 128, G4), (1, 0, 2))
    wihT = wihT.reshape(128, 2, 8, 128)[:, :, GPERM, :].reshape(128, 2, G4)
    wod = np.asarray(w_out, np.float32)[:, d * HD:(d + 1) * HD]
    woT = np.transpose(wod.T.reshape(2, 128, NT), (1, 0, 2))
    biasg = b_d.reshape(8, 128)[GPERM, :].T

    bs8 = slice(8 * c, 8 * c + 8)
    tg8 = np.asarray(tags[bs8], np.float32)
    tg = tg8.reshape(128, L)
    tgp8 = np.concatenate([np.full((8, 1), 16.0, np.float32), tg8[:, :-1]], 1)
    tgp = tgp8.reshape(128, L)
    mk8 = np.asarray(mask[bs8], np.float32)
    mk = mk8.reshape(128, L)
    mkz8 = mk8.copy()
    mkz8[:, 0] = 0.0
    mkz = mkz8.reshape(128, L).astype(np.int8)
    trans_f = np.asarray(trans, np.float32)
    trb = np.tile(trans_f.reshape(1, 16), (128, 1))
    trb2 = np.tile(trans_f.T.reshape(1, 16), (128, 1))
    stb = np.tile(_f(start_trans).reshape(1, NT), (128, 1))
    enb = np.tile(_f(end_trans).reshape(1, NT), (128, 1))
    bob = np.tile(_f(b_out).reshape(1, NT), (128, 1))
    jc = np.tile(np.arange(NT, dtype=np.float32).reshape(1, NT), (128, 1))
    gsel = np.zeros((128, 8), np.float32)
    for b in range(8):
        gsel[b * 16:(b + 1) * 16, b] = 1.0
    tag0 = np.asarray(tags[bs8, 0], np.float32).reshape(8, 1)
    lt = np.asarray(tags, np.int64)[np.arange(B), np.asarray(length, np.int64) - 1]
    ltag = np.asarray(lt[bs8], np.float32).reshape(8, 1)
    dirsel = np.full((4, 1), d, np.int8)
    scat = (np.arange(4, dtype=np.int32) * 4 + (c % 4)).reshape(4, 1)
    fidx = np.zeros((128, NT), np.int32)
    for b in range(8):
        b_abs = 8 * c + b
        for ch in range(CH):
            for j in range(NT):
                fidx[b * 16 + ch, j] = (j * 4 + b_abs // 16) * 256 + (b_abs % 16) * 16 + ch
    llsc = (8 * c + np.arange(8, dtype=np.int32)).reshape(8, 1)

    return {
        "embb": emb_bf, "gidx": gidx, "whhT": _bf(whhT), "wihT": _bf(wihT),
        "woT": _bf(woT), "biasg": _f(biasg), "mrhs": mrhs,
        "tg": _f(tg), "tgp": _f(tgp), "mk": _f(mk), "mkz": np.ascontiguousarray(mkz),
        "trb": _f(trb), "trb2": _f(trb2), "stb": _f(stb), "enb": _f(enb),
        "bob": _f(bob), "jc": _f(jc), "gsel": _f(gsel), "tag0": _f(tag0),
        "ltag": _f(ltag), "dirsel": np.ascontiguousarray(dirsel), "scat": scat, "fidx": fidx,
        "llsc": llsc,
    }


def kernel(sentence, tags, mask, length, embedding, w_ih_f, w_hh_f, b_f,
           w_ih_b, w_hh_b, b_b, w_out, b_out, start_trans, end_trans, trans):
    if "nc" not in _CACHED:
        _CACHED["nc"] = _build_program()
    nc = _CACHED["nc"]
    emb_bf = _bf(embedding)
    in_maps = [
        _prep_core_inputs(c, np.asarray(sentence), np.asarray(tags),
                          np.asarray(mask), np.asarray(length),
                          w_ih_f, w_hh_f, b_f, w_ih_b, w_hh_b, b_b,
                          w_out, b_out, start_trans, end_trans, trans, emb_bf)
        for c in range(NCORES)
    ]
    r = run_bass_kernel_spmd(nc, in_maps, core_ids=list(range(NCORES)))
    _CACHED["last_results"] = r
    return np.float32(r.results[0]["loss"].reshape(())[()])

